# revision 32
# baseline (speedup 1.0000x reference)
"""Trainium2 Bass kernel for nn_DescriptionEmbedding (attention-pooling).

Math: for each feature f, attention over W hidden words:
  score[f,w] = sum_h u[h] * tanh(a[f,h] + c[w,h]),  a = fe@W1, c = he@W2 + b
  attn = softmax_w(masked exp), context[f] = sum_w attn*he[w], out = values@context

Reformulation (exact identity + short series, j<=2; validated ~2.5e-3 with
bf16 operands):
  tanh(a+c) = ta + (1-ta^2)tc - (1-ta^2)ta tc^2 + ...
  S[w,f] = tc[w,:] @ P1[f,:].T + tc^2[w,:] @ P2[f,:].T
  P1 = u*(1-ta^2), P2 = -P1*ta   (the j=0 term cancels in softmax)

Per-core layout (F=2000 split 8 x 250, two halves of 125):
 - prep: ONE matmul per 1000-word tile with block-diag [[w2,0],[0,w2]] lhsT
   produces a [128,500] pre-activation; 128-wide tanh (bf16 out) halves ACT
   cost vs 64-wide. DVE squares + Pool copies assemble QT=[tc;tc^2] stacks.
 - score: [128,125]bf16 lhsT x PT[128,256]bf16 -> two 2-bank ps tiles per
   1000-word tile; exp on ACT in two halves (bf16 out) so the next tile's
   score matmuls overlap the second half; mask multiply on DVE (u8 mask).
 - context: lhsT=eq-chunk [125w,125f], rhs=heo [125w,17] -> ctx[125f,2,17]
   accumulated in PSUM across all 32 chunks: f-on-partition layout gives
   per-partition denominators (col 16) -> no transposes in the epilogue.
 - epilogue: reciprocal + per-partition scale -> values@context partial
   [16,256] per core; host sums the 8 partials.
Reps structure: U=8 bodies manually unrolled per For_i iteration (amortizes
the loop's all-engine barrier); two-stage software pipeline (head k emitted
before main k-1) plus double-buffered pools overlap DMA/prep with compute.
Inputs are dieted to ~1.45MB/core/rep (bf16 weights/values, u8 mask), all on
the SP HWDGE ring -- DMA bandwidth, not engine time, is the HW bottleneck.
"""
import os
import sys

import numpy as np

F, W, E, H, B = 2000, 4000, 16, 64, 256
NCORES = 8
FS = F // NCORES          # 250 features per core
FH = FS // 2              # 125: half-shard (partition dim of ctx)
PW = 125                  # words per chunk
NWC = W // PW             # 32 chunks
NT = 4                    # tiles (1000 words each)
CPT = NWC // NT           # 8 chunks per tile


def _import_concourse():
    # bass2jax executes via jax PJRT on the neuron devices; a cpu platform
    # pin would hide them. Clear it if jax hasn't been imported yet.
    if "jax" not in sys.modules and os.environ.get("JAX_PLATFORMS") == "cpu":
        del os.environ["JAX_PLATFORMS"]
    try:
        import concourse.bass  # noqa: F401
    except ImportError:
        for p in ("/opt/trn_rl_repo", os.path.expanduser("~/trn_rl_repo")):
            if os.path.isdir(p) and p not in sys.path:
                sys.path.insert(0, p)
        import concourse.bass  # noqa: F401


def build_nc(reps=1):
    _import_concourse()
    import concourse.bass as bass  # noqa: F401
    import concourse.mybir as mybir
    import concourse.tile as tile
    from concourse import bacc
    from concourse.alu_op_type import AluOpType
    from concourse.masks import make_identity

    f32 = mybir.dt.float32
    f32r = mybir.dt.float32r
    bf16 = mybir.dt.bfloat16
    u8 = mybir.dt.uint8
    ACT = mybir.ActivationFunctionType

    nc = bacc.Bacc(None, target_bir_lowering=False, debug=False)

    # scal cols (f32): 0=bb([b;b]), 1=-u, 2=+u, 3=pad
    # wf (bf16): cols 0:64=w1, 64:314=feT
    # bigbf (bf16): cols 0:1000=heT packed, 1000:1128=w2blk x2 bands
    scal = nc.dram_tensor("scal", [128, 4], f32, kind="ExternalInput")
    wf = nc.dram_tensor("wf", [E, 314], bf16, kind="ExternalInput")
    bigbf = nc.dram_tensor("bigbf", [64, 1128], bf16, kind="ExternalInput")
    heo = nc.dram_tensor("heo", [PW, NWC, 17], bf16, kind="ExternalInput")
    maskT = nc.dram_tensor("maskT", [PW, NWC, 256], u8, kind="ExternalInput")
    vT = nc.dram_tensor("vT", [FH, 2 * B], bf16, kind="ExternalInput")
    out = nc.dram_tensor("out", [E, B], f32, kind="ExternalOutput")

    import contextlib

    with tile.TileContext(nc) as tc:
        with (
            tc.tile_pool(name="boot", bufs=1) as boot,
            tc.tile_pool(name="consts", bufs=2) as consts,
            tc.tile_pool(name="rt", bufs=2) as rpool,
            tc.tile_pool(name="escore", bufs=2) as epool,
            tc.tile_pool(name="small", bufs=2) as small,
            tc.tile_pool(name="hp_ps", bufs=2, space="PSUM") as hp_ps,
            tc.tile_pool(name="s_ps", bufs=1, space="PSUM") as s_ps,
            tc.tile_pool(name="ctx_ps", bufs=1, space="PSUM") as ctx_ps,
        ):
            # dummy activations hoist the ACT table load out of the loop;
            # ident feeds the epilogue transposes (built once)
            dummy = boot.tile([1, 2], f32)
            nc.vector.memset(dummy[:], 0)
            nc.scalar.activation(dummy[:], dummy[:], ACT.Exp)
            nc.scalar.activation(dummy[:], dummy[:], ACT.Tanh)
            ident = boot.tile([17, 17], f32)
            make_identity(nc, ident[:])

            def epilogue(st):
                # normalize ctx (per-partition denominators) and produce the
                # [E, B] partial via values@context; emitted early in the
                # NEXT body so PE's prep matmuls aren't queued behind it
                pctx, vTs = st
                ctxT = small.tile([17, 256], f32, tag="ctxT")
                nc.vector.tensor_copy(ctxT[:, 0:FS], pctx[:, 0:FS])
                pth = [hp_ps.tile([FH, 17], f32, tag="hp", name=f"pt{h}")
                       for h in range(2)]
                for h in range(2):
                    nc.tensor.transpose(pth[h][:],
                                        ctxT[:, FH * h:FH * h + FH],
                                        ident[:])
                rv = small.tile([FH, 2], f32, tag="rv")
                for h in range(2):
                    nc.vector.reciprocal(rv[:, h:h + 1], pth[h][:, 16:17])
                ctxn = small.tile([FH, 2, E], bf16, tag="ctxn")
                for h in range(2):
                    nc.vector.tensor_scalar_mul(ctxn[:, h, :],
                                                pth[h][:, 0:E],
                                                rv[:, h:h + 1])
                po = hp_ps.tile([E, B], f32, tag="hp", name="po")
                for h in range(2):
                    nc.tensor.matmul(po[:], ctxn[:, h, :],
                                     vTs[:, B * h:B * h + B],
                                     start=(h == 0), stop=(h == 1))
                outsb = small.tile([E, B], f32, tag="outsb")
                nc.vector.tensor_copy(outsb[:], po[:])
                nc.gpsimd.dma_start(out[:], outsb[:])

            def head():
                """Input DMAs + PT prep + all prep tiles (tanh, QT builds).

                Emitted BEFORE the previous body's main-stage so PE's prep
                matmuls aren't queued behind the previous tail's ctx
                matmuls (which wait on the late mask multiplies).
                """
                # ---- input DMAs (all on the SP ring; an ACT-ring split
                # measured slower -- ACT dma_start issue cost)
                scals = consts.tile([128, 4], f32, name="scals")
                wfs = consts.tile([E, 314], bf16, name="wfs")
                bigs = consts.tile([64, 1128], bf16, name="bigs")
                mqs = consts.tile([PW, NWC, 256], u8, name="mqs")
                heos = consts.tile([PW, NWC, 17], bf16, name="heos")
                vTs = consts.tile([FH, 2 * B], bf16, name="vTs")
                nc.sync.dma_start(bigs[:], bigbf[:])
                nc.sync.dma_start(wfs[:], wf[:])
                nc.sync.dma_start(scals[:], scal[:])
                nc.sync.dma_start(mqs[:], maskT[:])
                nc.sync.dma_start(heos[:], heo[:])
                nc.sync.dma_start(vTs[:], vT[:])

                bbs = scals[:, 0:1]
                nus = scals[0:H, 1:2]   # -u
                pus = scals[0:H, 2:3]   # +u
                w1s = wfs[:, 0:64]
                feTs = wfs[:, 64:314]

                # ---- PT prep: PT[0:64]=u(1-ta^2), PT[64:128]=-PT1*ta ------
                pf = hp_ps.tile([H, FS], f32, tag="hp", name="pf")
                nc.tensor.matmul(pf[:], w1s, feTs, start=True, stop=True)
                ta = small.tile([H, FS], bf16, tag="ta")
                nc.scalar.activation(ta[:], pf[:], ACT.Tanh)
                PT = consts.tile([128, 256], bf16, name="PT")
                nc.vector.memset(PT[:, FS:256], 0)
                sq = small.tile([H, FS], bf16, tag="sq")
                nc.vector.tensor_tensor(sq[:], ta[:], ta[:], AluOpType.mult)
                # PT1 = (sq * -u) + u
                nc.vector.tensor_scalar(PT[0:H, 0:FS], sq[:], nus, pus,
                                        AluOpType.mult, AluOpType.add)
                # PT2 = (ta * -1) * PT1
                nc.vector.scalar_tensor_tensor(PT[H:128, 0:FS], ta[:], -1.0,
                                               PT[0:H, 0:FS], AluOpType.mult,
                                               AluOpType.mult)

                pctx = ctx_ps.tile([17, 256], f32, name="pctx")
                QTs = [consts.tile([128, 2 * 500], bf16, name=f"QT{t}")
                       for t in range(NT)]

                for t in range(NT):
                    # hp[0:64] = w2.T @ heT(words 1000t..+500); hp[64:128] =
                    # words +500..+1000 via block-diag w2blk (bias in tanh)
                    hp = hp_ps.tile([128, 500], f32, tag="hp", name=f"hp{t}")
                    bnd = 32 * (t // 2)
                    nc.tensor.matmul(hp[:], bigs[bnd:bnd + 32, 1000:1128],
                                     bigs[bnd:bnd + 32,
                                          500 * (t % 2):500 * (t % 2) + 500],
                                     start=True, stop=True)
                    r = rpool.tile([128, 500], bf16, tag="r", name=f"r{t}")
                    nc.scalar.activation(r[:], hp[:], ACT.Tanh, bias=bbs)
                    qt = QTs[t]
                    # tc rows: copies (Pool); tc^2 rows: squares (DVE)
                    nc.gpsimd.tensor_copy(qt[0:H, 0:500], r[0:H, :])
                    nc.gpsimd.tensor_copy(qt[0:H, 500:1000], r[H:128, :])
                    nc.vector.tensor_tensor(qt[H:128, 0:500], r[0:H, :],
                                            r[0:H, :], AluOpType.mult)
                    nc.vector.tensor_tensor(qt[H:128, 500:1000], r[H:128, :],
                                            r[H:128, :], AluOpType.mult)
                return (pctx, QTs, PT, mqs, heos, vTs)

            def main_stage(st):
                """Scores, exps, mask multiplies, ctx accumulation, epilogue
                for the body whose head() produced ``st``."""
                pctx, QTs, PT, mqs, heos, vTs = st
                for t in range(NT):
                    # two 2-bank ps tiles so next tile's score matmuls can
                    # start as soon as the matching exp half has drained
                    psh = [s_ps.tile([PW, CPT // 2, 256], f32, tag=f"ps{g}",
                                     name=f"ps{t}_{g}") for g in range(2)]
                    for i in range(CPT):
                        nc.tensor.matmul(psh[i // 4][:, i % 4, :],
                                         QTs[t][:, PW * i:PW * i + PW],
                                         PT[:], start=True, stop=True)
                    eq = epool.tile([PW, CPT, 256], bf16, name=f"eq{t}")
                    for g in range(2):
                        nc.scalar.activation(eq[:, 4 * g:4 * g + 4, :],
                                             psh[g][:], ACT.Exp)
                    nc.vector.tensor_tensor(eq[:], eq[:],
                                            mqs[:, CPT * t:CPT * t + CPT, :],
                                            AluOpType.mult)
                    for i in range(CPT):
                        wc = CPT * t + i
                        nc.tensor.matmul(pctx[:, 0:FS], heos[:, wc, :],
                                         eq[:, i, 0:FS],
                                         start=(wc == 0),
                                         stop=(wc == NWC - 1))
                epilogue((pctx, vTs))

            # Manual unroll: U bodies per For_i iteration so the loop's
            # all-engine barrier amortizes; two-stage software pipeline
            # (head of body k emitted before main of body k-1) keeps every
            # engine queue free of cross-body blocking.
            U = 8 if reps >= 8 else 1
            n_groups, rem = divmod(reps, U)

            def run_chain(n):
                st = head()
                for _ in range(n - 1):
                    st_next = head()
                    main_stage(st)
                    st = st_next
                main_stage(st)

            if n_groups >= 1:
                with tc.For_i(0, n_groups, 1):
                    run_chain(U)
            if rem:
                run_chain(rem)

    nc.compile()
    return nc


def shard_inputs(values, feature_emb, hidden_emb, W_w, b_w, W_u, mask):
    """Host-side shard/layout prep. Returns per-core input maps."""
    from ml_dtypes import bfloat16

    values = np.ascontiguousarray(values, dtype=np.float32)
    fe = np.ascontiguousarray(feature_emb, dtype=np.float32)
    he = np.ascontiguousarray(hidden_emb, dtype=np.float32)
    W_w = np.ascontiguousarray(W_w, dtype=np.float32)
    b_w = np.ascontiguousarray(b_w, dtype=np.float32)
    W_u = np.ascontiguousarray(W_u, dtype=np.float32)
    m = np.asarray(mask).reshape(F, W)

    w1 = W_w[:E]                                          # [E, H]
    w2 = W_w[E:]                                          # [E, H]
    w2blk = np.zeros((32, 128), np.float32)
    w2blk[0:E, 0:H] = w2
    w2blk[E:32, H:128] = w2

    heT_full = he.T                                       # [E, W]
    # [64, 1000]: tile t at rows 32*(t//2), cols 500*(t%2); each tile's two
    # 500-word halves stacked 16+16 on the partition dim (block-diag w2blk)
    heT = np.zeros((64, 1000), np.float32)
    for t in range(NT):
        r0, c0 = 32 * (t // 2), 500 * (t % 2)
        heT[r0:r0 + 16, c0:c0 + 500] = heT_full[:, 1000 * t:1000 * t + 500]
        heT[r0 + 16:r0 + 32, c0:c0 + 500] = \
            heT_full[:, 1000 * t + 500:1000 * t + 1000]

    heo_flat = np.concatenate([he, np.ones((W, 1), np.float32)], axis=1)
    heo = np.ascontiguousarray(
        heo_flat.reshape(NWC, PW, 17).transpose(1, 0, 2)).astype(bfloat16)

    # bigbf [64, 1128] bf16: cols 0:1000 heT packed, 1000:1128 w2blk bands
    bigbf = np.zeros((64, 1128), bfloat16)
    bigbf[:, 0:1000] = heT.astype(bfloat16)
    bigbf[:, 1000:1128] = np.tile(w2blk, (2, 1)).astype(bfloat16)

    scal = np.zeros((128, 4), np.float32)
    scal[:, 0] = np.concatenate([b_w, b_w])
    scal[0:H, 1] = -W_u[:, 0]
    scal[0:H, 2] = W_u[:, 0]

    mT_full = m.T.astype(np.uint8)                        # [W, F]
    vT_full = values.astype(bfloat16).T                   # [F, B]
    feT_full = fe.T                                       # [E, F]

    in_maps = []
    for c in range(NCORES):
        sl = slice(c * FS, (c + 1) * FS)
        wfd = np.zeros((E, 314), bfloat16)
        wfd[:, 0:64] = w1.astype(bfloat16)
        wfd[:, 64:314] = feT_full[:, sl].astype(bfloat16)
        mq = np.zeros((PW, NWC, 256), np.uint8)
        mq[:, :, :FS] = mT_full[:, sl].reshape(NWC, PW, FS).transpose(1, 0, 2)
        vt = np.ascontiguousarray(
            vT_full[sl].reshape(2, FH, B).transpose(1, 0, 2).reshape(
                FH, 2 * B))
        in_maps.append({
            "scal": scal, "wf": wfd, "bigbf": bigbf,
            "heo": heo,
            "maskT": mq,
            "vT": vt,
        })
    return in_maps


_CACHED = {}


def kernel(values, feature_emb, hidden_emb, W_w, b_w, W_u, mask):
    _import_concourse()
    from concourse.bass_utils import run_bass_kernel_spmd

    if "nc" not in _CACHED:
        _CACHED["nc"] = build_nc()
    nc = _CACHED["nc"]
    in_maps = shard_inputs(values, feature_emb, hidden_emb, W_w, b_w, W_u, mask)
    res = run_bass_kernel_spmd(nc, in_maps, list(range(NCORES)))
    parts = [res.results[c]["out"] for c in range(NCORES)]
    acc = np.sum(np.stack(parts, 0), 0, dtype=np.float32)   # [E, B]
    return np.ascontiguousarray(acc.T)
